# revision 3
# baseline (speedup 1.0000x reference)
"""Single-head causal attention (B=8, T=2048, C=1024, H=128) on 8 TRN2 NeuronCores.

Strategy: pure data-parallel over batch — one batch element per core, zero
collectives.  v2 schedule: keep the PE continuously busy from the framework
start barrier (~6.8us) to the last PV chain:

  - warmup filler matmuls on a memset tile start at the barrier so the HAM
    clock-gate releases (1.2 -> 2.4 GHz) ~3.4us in, and the PE has work
    during the input-DMA latency window.
  - input DMAs are depth-limited (tiny same-engine stall reads) so each DMA
    ring finishes tiles in demand order instead of round-robining them all
    (RR makes the first tile land as late as the last).  hh=0 tiles split
    across the scalar+sync HW-DGE rings; hh=1 tiles whole on the gpsimd
    SW-DGE ring (its ~3us startup is hidden; they're needed late).
  - projection emitted per 512-col segment (cc-inner); q/k/v drains all on
    DVE (drain order k,q,v) so ACT does nothing but exp; S(j) units weave
    into the next projection segment's matmuls at ~2 S units per 2 chunk
    triples so exp (ACT) streams while the PE keeps projecting.
  - S units are single k-tiles [128(k), 512(q)] in one PSUM bank: matmul
    only the causal column range, exp only that range, triangle mask on
    GpSimd for diagonal tiles.  PSUM: 3 proj-acc banks (reused by PV accum
    tiles at the tail) + 4 S banks + 1 warmup/transpose bank = 8.
  - all PV chains run at the tail, woven with S3 units so PV0-2 fill the
    PE while ACT streams S3's exps; PV3 (the only exp-gated work) starts
    right as its exps complete.  out_aug ones-column gives the softmax
    denominator; DVE reciprocal+scale; one output DMA per 128-row q tile.
"""

import numpy as np
import ml_dtypes

import concourse.bass as bass  # noqa: F401
import concourse.mybir as mybir
import concourse.tile as tile
from concourse import bacc
from concourse.bass_utils import run_bass_kernel_spmd

B, T, C, H = 8, 2048, 1024, 128
NCORES = 8
P = 128
SEG = 512
BF16 = mybir.dt.bfloat16
F32 = mybir.dt.float32
SCALE = float(C) ** -0.5

NFILL = 12  # warmup filler matmuls (cover barrier -> first-input latency)

LAST_RESULT = None


def build_nc(t=T, reps=1):
    nchunk = C // P      # 8
    ntile = t // P       # 16 k-tiles
    nblk = t // SEG      # 4 q-blocks
    tpb = SEG // P       # 4 q-tiles per block
    nhalf = max(1, t // 1024)
    hw = t // nhalf      # 1024

    nc = bacc.Bacc("TRN2", target_bir_lowering=False, debug=False)

    npair = (C // P) // 2
    xt_d = nc.dram_tensor("xt", [2, npair, P, 2, t // 2], BF16,
                          kind="ExternalInput")
    w3_d = nc.dram_tensor("w3", [P, nchunk, 3, H], BF16, kind="ExternalInput")
    b3_d = nc.dram_tensor("b3", [H, 3], F32, kind="ExternalInput")
    te_d = nc.dram_tensor("te", [P, 2 * P], BF16, kind="ExternalInput")
    out_d = nc.dram_tensor("out", [t, H], F32, kind="ExternalOutput")

    Exp = mybir.ActivationFunctionType.Exp
    n_es = (nblk * (nblk + 1) // 2) * tpb  # total S units

    with tile.TileContext(nc) as tc:
        with (
            tc.tile_pool(name="const", bufs=1) as cpool,
            tc.tile_pool(name="big", bufs=1) as bpool,
            tc.tile_pool(name="v", bufs=ntile) as vpool,
            tc.tile_pool(name="es", bufs=n_es) as espool,
            tc.tile_pool(name="o", bufs=1) as opool,
            tc.tile_pool(name="ps", bufs=1, space="PSUM") as pspool,
        ):
          for rep in range(reps):
            # ---- PE warmup fillers (deps: one DVE memset only)
            warm_s = cpool.tile([P, SEG], BF16, tag="warm", name="warm_t")
            nc.vector.memset(warm_s[:], 0.0)
            warm_ps = pspool.tile([P, 2, H + 32], F32, tag="pso", bufs=1,
                                  name="warm_ps")
            for _ in range(NFILL):
                nc.tensor.matmul(warm_ps[:, 0, 0:H], warm_s[:, 0:P],
                                 warm_s[:, P:P + H], start=True, stop=True)

            # ---- input tiles
            xt_s = {(pp, hh): cpool.tile([P, 2, t // 2], BF16,
                                         tag=f"xtp{pp}_{hh}",
                                         name=f"xtp{pp}_{hh}")
                    for pp in range(npair) for hh in range(2)}
            w_s = cpool.tile([P, nchunk, 3, H], BF16, tag="w3", name="w3_t")
            te_s = cpool.tile([P, 2 * P], BF16, tag="te", name="te_t")
            b_s = cpool.tile([P, 3], F32, tag="b3", name="b3_t")
            fl_s = cpool.tile([P, 8], BF16, tag="flow", name="flow_t")
            tri_s = te_s[:, 0:P]
            eye_s = te_s[:, P:2 * P]

            # ---- input DMAs, demand-ordered with depth-2 flow control
            HB = P // 2
            # scalar ring: bottom halves of hh=0 pairs
            nc.scalar.dma_start(out=xt_s[(0, 0)][0:HB], in_=xt_d[0, 0, 0:HB])
            nc.scalar.dma_start(out=xt_s[(1, 0)][0:HB], in_=xt_d[0, 1, 0:HB])
            nc.scalar.dma_start(out=fl_s[0:1, 0:1],
                                in_=xt_s[(0, 0)][0:1, 0:1, 0:1])
            nc.scalar.dma_start(out=xt_s[(2, 0)][0:HB], in_=xt_d[0, 2, 0:HB])
            nc.scalar.dma_start(out=fl_s[0:1, 1:2],
                                in_=xt_s[(1, 0)][0:1, 0:1, 0:1])
            nc.scalar.dma_start(out=xt_s[(3, 0)][0:HB], in_=xt_d[0, 3, 0:HB])
            # sync ring: weights + top halves of hh=0 pairs + small consts
            nc.sync.dma_start(out=w_s[:, 0:nchunk // 2],
                              in_=w3_d[:, 0:nchunk // 2])
            nc.sync.dma_start(out=xt_s[(0, 0)][HB:P], in_=xt_d[0, 0, HB:P])
            nc.sync.dma_start(out=w_s[:, nchunk // 2:nchunk],
                              in_=w3_d[:, nchunk // 2:nchunk])
            nc.sync.dma_start(out=xt_s[(1, 0)][HB:P], in_=xt_d[0, 1, HB:P])
            nc.sync.dma_start(out=fl_s[HB:HB + 1, 2:3],
                              in_=xt_s[(0, 0)][HB:HB + 1, 0:1, 0:1])
            nc.sync.dma_start(out=xt_s[(2, 0)][HB:P], in_=xt_d[0, 2, HB:P])
            nc.sync.dma_start(out=fl_s[HB:HB + 1, 3:4],
                              in_=xt_s[(1, 0)][HB:HB + 1, 0:1, 0:1])
            nc.sync.dma_start(out=xt_s[(3, 0)][HB:P], in_=xt_d[0, 3, HB:P])
            nc.sync.dma_start(out=b_s[:], in_=b3_d[:])
            nc.sync.dma_start(out=te_s[:], in_=te_d[:])
            # gpsimd ring: whole hh=1 pairs
            nc.gpsimd.dma_start(out=xt_s[(0, 1)][:], in_=xt_d[1, 0])
            nc.gpsimd.dma_start(out=xt_s[(1, 1)][:], in_=xt_d[1, 1])
            nc.gpsimd.dma_start(out=fl_s[0:1, 4:5],
                                in_=xt_s[(0, 1)][0:1, 0:1, 0:1])
            nc.gpsimd.dma_start(out=xt_s[(2, 1)][:], in_=xt_d[1, 2])
            nc.gpsimd.dma_start(out=fl_s[0:1, 5:6],
                                in_=xt_s[(1, 1)][0:1, 0:1, 0:1])
            nc.gpsimd.dma_start(out=xt_s[(3, 1)][:], in_=xt_d[1, 3])

            qt_s = bpool.tile([P, t], BF16, tag="qt", name="qt_t")
            kt_s = bpool.tile([P, t], BF16, tag="kt", name="kt_t")
            vt_s = bpool.tile([P, t], BF16, tag="vt", name="vt_t")
            v_s = [None] * ntile
            W = {"wq": 0, "wk": 1, "wv": 2}
            NAMES = ("wq", "wk", "wv")
            DST = {"wq": qt_s, "wk": kt_s, "wv": vt_s}

            acc = {}

            def proj_open(hf, s2):
                for name in NAMES:
                    acc[name] = pspool.tile([P, SEG], F32, tag="acc", bufs=3,
                                            name=f"acc_{name}_{hf}_{s2}")

            def proj_mms(hf, s2, cc_list):
                for cc in cc_list:
                    for name in NAMES:
                        nc.tensor.matmul(
                            acc[name][:],
                            w_s[:, cc, W[name], :],
                            xt_s[(cc // 2, hf)][:, cc % 2,
                                                s2 * SEG:(s2 + 1) * SEG],
                            start=(cc == 0), stop=(cc == nchunk - 1),
                        )

            def proj_drain(hf, s2):
                base = hf * hw + s2 * SEG
                for name in ("wk", "wq", "wv"):  # k first (gates S), q, v
                    nc.vector.tensor_scalar_add(
                        DST[name][:, base:base + SEG], acc[name][:],
                        b_s[:, W[name]:W[name] + 1])

            def vtrans2(m0):
                # two transposes share one 2-slot PSUM tile so they don't
                # serialize on the single pso bank
                pst = pspool.tile([P, 2, P], BF16, tag="pso", bufs=1,
                                  name=f"pst{m0}")
                for u in (0, 1):
                    m = m0 + u
                    nc.tensor.transpose(pst[:, u],
                                        vt_s[:, m * P:(m + 1) * P], eye_s)
                    v = vpool.tile([P, H + 1], BF16, tag="v", name=f"vtile{m}")
                    nc.vector.tensor_copy(v[:, 0:H], pst[:, u])
                    nc.vector.memset(v[:, H:H + 1], 1.0)
                    v_s[m] = v

            es_all = {}

            def S_unit(j, m):
                es_of = es_all.setdefault(j, [None] * (tpb * j + tpb))
                r = m - tpb * j
                off = P * r if r > 0 else 0
                ps = pspool.tile([P, SEG], F32, tag="spsum", bufs=4,
                                 name=f"sps{j}_{m}")
                es = espool.tile([P, SEG], BF16, tag="es", name=f"es{j}_{m}")
                nc.tensor.matmul(
                    ps[:, off:SEG],
                    kt_s[:, m * P:(m + 1) * P],
                    qt_s[:, j * SEG + off:(j + 1) * SEG],
                    start=True, stop=True,
                )
                nc.scalar.activation(es[:, off:SEG], ps[:, off:SEG], Exp,
                                     scale=SCALE)
                if r >= 0:
                    # diagonal tile: triangle mask on the [128,128] block
                    nc.gpsimd.tensor_mul(
                        es[:, off:off + P], es[:, off:off + P], tri_s)
                es_of[m] = es

            pts = [None, None]
            chain_no = [0]

            def pv_open():
                # PV accumulators reuse the (now dead) proj-acc banks
                pts[0] = pspool.tile([P, 2, H + 32], F32, tag="acc", bufs=3,
                                     name="pv_ps0")
                pts[1] = pspool.tile([P, 2, H + 32], F32, tag="acc", bufs=3,
                                     name="pv_ps1")

            obs = {}

            def pv_ob(j):
                obs[j] = opool.tile([P, tpb, H], F32, tag="ob", bufs=2,
                                    name=f"ob{j}")

            def PV_chain(j, rr):
                es_of = es_all[j]
                ob = obs[j]
                i = tpb * j + rr
                cn = chain_no[0]
                chain_no[0] += 1
                pso = pts[cn % 2][:, (cn // 2) % 2, 0:H + 1]
                for m in range(i + 1):
                    nc.tensor.matmul(
                        pso[:],
                        es_of[m][:, rr * P:rr * P + P],
                        v_s[m][:],
                        start=(m == 0), stop=(m == i),
                    )
                rc = opool.tile([P, 1], F32, tag="rc", bufs=4, name=f"rc{i}")
                nc.vector.reciprocal(rc[:], pso[:, H:H + 1])
                nc.vector.tensor_scalar_mul(ob[:, rr, :], pso[:, 0:H], rc[:])
                q = (nc.sync, nc.gpsimd, nc.scalar)[i % 3]
                q.dma_start(out=out_d[i * P:(i + 1) * P, :], in_=ob[:, rr, :])

            if t >= 2048:
                proj_open(0, 0)
                proj_mms(0, 0, range(nchunk))
                proj_drain(0, 0)
                S_unit(0, 0); S_unit(0, 1)
                proj_open(0, 1)
                proj_mms(0, 1, range(0, 2))
                vtrans2(0)
                proj_mms(0, 1, range(2, 4))
                S_unit(0, 2); S_unit(0, 3)
                vtrans2(2)
                proj_mms(0, 1, range(4, 6))
                proj_mms(0, 1, range(6, 8))
                proj_drain(0, 1)
                S_unit(1, 0); S_unit(1, 1)
                proj_open(1, 0)
                proj_mms(1, 0, range(0, 2))
                S_unit(1, 2); S_unit(1, 3)
                proj_mms(1, 0, range(2, 4))
                vtrans2(4)
                S_unit(1, 4); S_unit(1, 5)
                proj_mms(1, 0, range(4, 6))
                vtrans2(6)
                S_unit(1, 6); S_unit(1, 7)
                proj_mms(1, 0, range(6, 8))
                proj_drain(1, 0)
                S_unit(2, 0); S_unit(2, 1)
                proj_open(1, 1)
                proj_mms(1, 1, range(0, 2))
                S_unit(2, 2); S_unit(2, 3)
                proj_mms(1, 1, range(2, 4))
                vtrans2(8)
                S_unit(2, 4); S_unit(2, 5)
                proj_mms(1, 1, range(4, 6))
                vtrans2(10)
                S_unit(2, 6); S_unit(2, 7)
                proj_mms(1, 1, range(6, 8))
                proj_drain(1, 1)
                S_unit(2, 8); S_unit(2, 9)
                S_unit(2, 10); S_unit(2, 11)
                # ---- tail: S3 woven with all PV work
                S_unit(3, 0); S_unit(3, 1)
                vtrans2(12)
                S_unit(3, 2); S_unit(3, 3)
                vtrans2(14)
                pv_open()
                pv_ob(0); pv_ob(1)
                S_unit(3, 4)
                PV_chain(0, 0); PV_chain(0, 1)
                S_unit(3, 5)
                PV_chain(0, 2); PV_chain(0, 3)
                S_unit(3, 6)
                PV_chain(1, 0)
                S_unit(3, 7)
                PV_chain(1, 1)
                S_unit(3, 8)
                PV_chain(1, 2)
                S_unit(3, 9)
                PV_chain(1, 3)
                pv_ob(2)
                S_unit(3, 10)
                PV_chain(2, 0)
                S_unit(3, 11)
                PV_chain(2, 1)
                S_unit(3, 12)
                PV_chain(2, 2)
                S_unit(3, 13)
                PV_chain(2, 3)
                S_unit(3, 14); S_unit(3, 15)
                pv_ob(3)
                PV_chain(3, 0)
                PV_chain(3, 1)
                PV_chain(3, 2)
                PV_chain(3, 3)
            else:
                for hf in range(nhalf):
                    for s2 in range(hw // SEG):
                        proj_open(hf, s2)
                        proj_mms(hf, s2, range(nchunk))
                        proj_drain(hf, s2)
                for m0 in range(0, ntile, 2):
                    vtrans2(m0)
                pv_open()
                for j in range(nblk):
                    for m in range(tpb * j + tpb):
                        S_unit(j, m)
                    pv_ob(j)
                    for rr in range(tpb):
                        PV_chain(j, rr)

    nc.finalize()
    return nc


_NC_CACHE = {}


def _get_nc(t=T, reps=1):
    key = (t, reps)
    if key not in _NC_CACHE:
        _NC_CACHE[key] = build_nc(t, reps)
    return _NC_CACHE[key]


def make_in_maps(embedded_data, Wq, bq, Wk, bk, Wv, bv, t=T):
    bf = ml_dtypes.bfloat16
    tri = np.triu(np.ones((P, P), dtype=np.float32))  # tri[k,q]=1 iff q>=k
    eye = np.eye(P, dtype=np.float32)
    te = np.concatenate([tri, eye], axis=1).astype(bf)
    w3 = np.stack([np.asarray(w, np.float32) for w in (Wq, Wk, Wv)])  # [3,C,H]
    # pre-transpose to [P, C//P, 3, H] so the DMA is contiguous per partition
    w3 = np.ascontiguousarray(
        w3.reshape(3, C // P, P, H).transpose(2, 1, 0, 3)).astype(bf)
    b3 = np.stack(
        [np.asarray(x, np.float32).reshape(H) for x in (bq, bk, bv)], axis=1)
    shared = {"w3": w3, "b3": np.ascontiguousarray(b3), "te": te}
    in_maps = []
    for b in range(NCORES):
        m = dict(shared)
        xtf = np.asarray(embedded_data[b], np.float32).T[:, :t]  # [C, t]
        # [pp, e, p, h, col] -> [h, pp, p, e, col]
        arr = xtf.reshape(C // P // 2, 2, P, 2, t // 2).transpose(3, 0, 2, 1, 4)
        m["xt"] = np.ascontiguousarray(arr).astype(bf)
        in_maps.append(m)
    return in_maps


def kernel(embedded_data, Wq, bq, Wk, bk, Wv, bv, trace=False):
    global LAST_RESULT
    nc = _get_nc(T)
    in_maps = make_in_maps(embedded_data, Wq, bq, Wk, bk, Wv, bv, T)
    res = run_bass_kernel_spmd(nc, in_maps, core_ids=list(range(NCORES)), trace=trace)
    LAST_RESULT = res
    out = np.stack([np.asarray(res.results[i]["out"]) for i in range(NCORES)])
    return out.astype(np.float32)
